# revision 33
# baseline (speedup 1.0000x reference)
"""Trainium2 Bass kernel for nn_ConvCombAttention (dense_transformer).

Sharding: pure data parallel — 16 samples split as 2 per NeuronCore across 8 cores.
All weights are host-folded (BN into depthwise conv, GroupNorm affine + softmax
scale into the 1x1 projections) and replicated.

Device algorithm per core (2 samples; bf16 GEMM datapath, f32 accumulation and
statistics):
  - depthwise 7x7/s4 conv as 49 PSUM-accumulated bf16 matmuls with diagonal lhsT
    (diagonals built on-device by gpsimd.affine_select from compact weights);
    x is downcast once to a margined bf16 working copy that also serves as the
    phase-C residual source
  - GroupNorm(1 group) via matmul cross-partition reductions + per-partition affine
  - channel attention computed entirely in S^T layout (no transposes anywhere);
    softmax denominators via ones-matmuls, normalization folded into PSUM evictions
  - spatial MHA with K=32 row-tiled matmuls and 32-wide col-tiled vals/denom matmuls
  - transposed depthwise conv (8x8/s4) as 64 diag bf16 matmuls + an identity
    65th tap that adds the x residual inside the PSUM accumulation; evictions
    alternate DVE/Act so the strided interleave writes split across engines

Active variant ('fullv4'): weight + output DMAs issue from the Activation-engine
DGE queue (parallel to x input DMAs on SP), the x downcast splits across
Act/DVE, and the spatial-softmax loop is software-pipelined with lookahead 2
(samples interleaved) so the in-order PE queue never stalls on the Act exp.
"""
import sys, os
for _p in ('/opt/trn_rl_repo', '/root/.axon_site/_ro/trn_rl_repo'):
    if os.path.isdir(_p) and _p not in sys.path:
        sys.path.append(_p)

import numpy as np
import concourse.bass as bass
import concourse.mybir as mybir
import concourse.tile as tile
from concourse import bacc
from contextlib import ExitStack

f32 = mybir.dt.float32
f32r = mybir.dt.float32r
bf16 = mybir.dt.bfloat16
Alu = mybir.AluOpType
Act = mybir.ActivationFunctionType

C = 384; HEADS = 12; HD = 32
EPS = 1e-5
H = W = 16; L = H * W          # downsampled spatial
NS = 2                          # samples per core
NCORES = 8
P = 128; NCH = C // P           # channel chunks
CUT = None                      # debug: 'nodma' | 'noevict' | 'nomm'
CL = np.float32(C * L)          # groupnorm normalizer


# ----------------------------------------------------------------------------- host fold
def host_precompute(inp):
    d = {}
    import ml_dtypes
    s = (inp['bn_gamma'] / np.sqrt(inp['bn_var'] + EPS)).astype(np.float32)
    dw = (inp['dw_w'].reshape(C, 49) * s[:, None]).astype(np.float32)
    # bf16: the whole depthwise-conv path (diagonals, x) runs in bf16 (PE 2x)
    d['dw_w'] = dw.reshape(NCH, P, 49).astype(ml_dtypes.bfloat16).copy()
    d['bn_b'] = (inp['bn_beta'] - inp['bn_mean'] * s).astype(np.float32).reshape(NCH, P, 1).copy()

    Wg1 = (inp['pwise_ch_w'] * inp['lnch_gamma'][None, :]).astype(np.float32)
    Wb1 = (inp['pwise_ch_w'] @ inp['lnch_beta']).astype(np.float32)
    sc1 = np.float32(1.0 / np.sqrt(np.sqrt(np.float32(L))))
    d['wqkch'] = np.concatenate([Wg1[0:C] * sc1, Wg1[C:2*C] * sc1], 0).T.reshape(
        NCH, P, 2*C).astype(ml_dtypes.bfloat16).copy()
    d['wbqkch_row'] = (np.concatenate([Wb1[0:C], Wb1[C:2*C]]) * sc1)[None, :].astype(np.float32).copy()
    d['wvch'] = Wg1[2*C:3*C].T.copy().reshape(NCH, P, C).astype(ml_dtypes.bfloat16).copy()
    d['wbvch'] = Wb1[2*C:3*C].astype(np.float32).reshape(NCH, P, 1).copy()
    d['outch'] = inp['outch_w'].T.copy().reshape(NCH, P, C).astype(ml_dtypes.bfloat16).copy()
    d['outchb'] = inp['outch_b'].astype(np.float32).reshape(NCH, P, 1).copy()

    Wg2 = (inp['pwise_sp_w'] * inp['lnsp_gamma'][None, :]).astype(np.float32)
    Wb2 = (inp['pwise_sp_w'] @ inp['lnsp_beta']).astype(np.float32)
    sc2 = np.float32(1.0 / np.sqrt(np.sqrt(np.float32(HD))))
    qrows = np.concatenate([np.arange(h*3*HD, h*3*HD+HD) for h in range(HEADS)])
    krows = qrows + HD
    vrows = qrows + 2*HD
    d['wqksp'] = np.concatenate([Wg2[qrows] * sc2, Wg2[krows] * sc2], 0).T.reshape(
        NCH, P, 2*C).astype(ml_dtypes.bfloat16).copy()
    d['wbqksp'] = (np.concatenate([Wb2[qrows], Wb2[krows]]) * sc2).astype(np.float32).reshape(2*NCH, P, 1).copy()
    d['wvsp'] = Wg2[vrows].T.copy().reshape(NCH, P, C).astype(ml_dtypes.bfloat16).copy()
    d['wbvsp_row'] = Wb2[vrows][None, :].astype(np.float32).copy()
    # [32(d), 12(h)*384(oc)]: per-head K=32 blocks, partition base 0; bf16 (SBUF)
    import ml_dtypes
    d['outsp'] = inp['outsp_w'].T.reshape(HEADS, HD, C).transpose(1, 0, 2).reshape(
        HD, HEADS*C).astype(ml_dtypes.bfloat16).copy()
    d['outspb'] = inp['outsp_b'].astype(np.float32).reshape(NCH, P, 1).copy()

    # transposed depthwise conv: out[c,4a+r1,4b+r2] = sum_t K[c,(r1,r2),t] x3p[c,a+j1,b+j2]
    # stored bf16: the up-conv path (updiag, x3p) runs in bf16
    wf = inp['up_w'][:, 0, ::-1, ::-1].astype(np.float32)
    taps = {r: [(j, 5 - r + 4*j) for j in (-1, 0, 1) if 0 <= 5 - r + 4*j < 8] for r in range(4)}
    # col 64 = 1.0: identity tap, used to add the x residual inside the PSUM
    # accumulation (kills the ofull prefill copies)
    upw = np.zeros((C, 65), np.float32)
    upw[:, 64] = 1.0
    upoff = []
    for r1 in range(4):
        for r2 in range(4):
            lst = []
            t = 0
            for (j1, m1) in taps[r1]:
                for (j2, m2) in taps[r2]:
                    upw[:, (r1*4+r2)*4 + t] = wf[:, m1, m2]
                    lst.append((j1, j2))
                    t += 1
            upoff.append(lst)
    d['upw'] = upw.reshape(NCH, P, 65).astype(ml_dtypes.bfloat16).copy()
    d['upoff'] = upoff    # python-side constant (16 x 4 (j1,j2))
    d['upb'] = inp['up_b'].astype(np.float32).reshape(NCH, P, 1).copy()

    # v2 variant: host-expanded diagonal weight tensors (DMA'd instead of
    # built on-device with gpsimd.affine_select). Layout [NCH, P, T, P] with
    # diag[c, p, t, q] = w[c, p, t] * (p == q).
    dwf = np.asarray(d['dw_w'], np.float32)            # [NCH, P, 49]
    dd = np.zeros((NCH, P, 70, P), np.float32)
    eye = np.eye(P, dtype=np.float32)
    for cc in range(NCH):
        dd[cc, :, 0:49, :] = dwf[cc][:, :, None] * eye[:, None, :]
        neg = -dwf[cc].reshape(P, 7, 7)[:, :, 0:3].reshape(P, 21)
        dd[cc, :, 49:70, :] = neg[:, :, None] * eye[:, None, :]
    d['dwdiag'] = dd.astype(ml_dtypes.bfloat16).copy()
    upf = np.asarray(d['upw'], np.float32)             # [NCH, P, 65]
    ud = upf[:, :, :, None] * eye[None, :, None, :]
    d['updiagh'] = ud.astype(ml_dtypes.bfloat16).copy()
    return d


DEV_KEYS = ['dw_w', 'bn_b', 'wqkch', 'wbqkch_row', 'wvch', 'wbvch', 'outch', 'outchb',
            'wqksp', 'wbqksp', 'wvsp', 'wbvsp_row', 'outsp', 'outspb', 'upw', 'upb',
            'dwdiag', 'updiagh']


# ----------------------------------------------------------------------------- device program
AB_KEYS = {'wqkch', 'wbqkch_row', 'wvch', 'wbvch', 'outch', 'outchb',
           'wqksp', 'wbqksp', 'wvsp', 'wbvsp_row', 'outsp', 'outspb', 'updiagh'}


def emit_isolate(nc, tc, ctx, io, upoff, stage):
    """Timing-isolation stages: run one suspect section on dummy data."""
    wp = ctx.enter_context(tc.tile_pool(name="wp", bufs=1))
    ps = ctx.enter_context(tc.tile_pool(name="ps", bufs=4, space="PSUM"))
    mm = nc.tensor.matmul
    upwt = []
    for c in range(NCH):
        w = wp.tile([P, 65], bf16, tag=f"upw{c}", name=f"upw{c}")
        nc.sync.dma_start(out=w, in_=io['upw'][c])
        upwt.append(w)
    upbt = []
    for c in range(NCH):
        w = wp.tile([P, 1], f32, tag=f"upb{c}", name=f"upb{c}")
        nc.sync.dma_start(out=w, in_=io['upb'][c])
        upbt.append(w)
    with ExitStack() as c1:
        pp = c1.enter_context(tc.tile_pool(name="pp", bufs=1))
        sC = c1.enter_context(tc.tile_pool(name="sC", bufs=2))
        sCo = c1.enter_context(tc.tile_pool(name="sCo", bufs=3))
        xbf = [pp.tile([P, NS, 4096], bf16, tag=f"xbf{c}", name=f"xbf{c}") for c in range(NCH)]
        x3p = [pp.tile([P, NS, 18, 18], bf16, tag=f"x3p{c}", name=f"x3p{c}") for c in range(NCH)]
        for c in range(NCH):
            nc.vector.memset(xbf[c], 0.5)
            nc.vector.memset(x3p[c], 0.5)
        if stage == 'selonly':
            # 3 bf16 affine_selects only
            for cc in range(NCH):
                ud = sC.tile([P, 65, P], bf16, tag="diag", name=f"ud{cc}")
                nc.gpsimd.affine_select(
                    out=ud, in_=upwt[cc].unsqueeze(2).broadcast_to([P, 65, P]),
                    compare_op=Alu.is_equal, fill=0.0, base=0,
                    pattern=[[0, 65], [1, P]], channel_multiplier=-1)
                sc = sC.tile([P, 64], f32, tag="sc", name=f"sc{cc}")
                nc.scalar.copy(out=sc, in_=ud[:, 0:64, 0])
                nc.sync.dma_start(out=io['out'][0, cc*P:(cc+1)*P].rearrange(
                    "c h w -> c (h w)")[:, 0:64], in_=sc)
            return
        # 'conly': full phase C structure on dummy data
        for cc in range(NCH):
            ud = sC.tile([P, 65, P], bf16, tag="diag", name=f"ud{cc}")
            nc.gpsimd.affine_select(
                out=ud, in_=upwt[cc].unsqueeze(2).broadcast_to([P, 65, P]),
                compare_op=Alu.is_equal, fill=0.0, base=0,
                pattern=[[0, 65], [1, P]], channel_multiplier=-1)
            xbf4 = xbf[cc].rearrange("p s (h w) -> p s h w", h=64)
            ofull = [sCo.tile([P, 4096], f32, tag="ofull", name=f"of{cc}_{s}")
                     for s in range(NS)]
            of4 = [o.rearrange("p (a b) -> p a b", a=64) for o in ofull]
            for r1 in range(4):
                for r2 in range(4):
                    ph = r1*4 + r2
                    acc = ps.tile([P, NS, 16, 16], f32, tag="bank", name="upps")
                    for t in range(4):
                        j1, j2 = upoff[ph][t]
                        mm(acc, lhsT=ud[:, ph*4+t, :],
                           rhs=x3p[cc][:, :, 1+j1:17+j1, 1+j2:17+j2],
                           start=(t == 0), stop=(t == 3))
                    for s in range(NS):
                        nc.vector.scalar_tensor_tensor(
                            out=of4[s][:, r1::4, r2::4], in0=acc[:, s],
                            scalar=upbt[cc][:, 0:1],
                            in1=xbf4[:, s, r1::4, r2::4],
                            op0=Alu.add, op1=Alu.add)
            for s in range(NS):
                nc.sync.dma_start(out=io['out'][s, cc*P:(cc+1)*P].rearrange("c h w -> c (h w)"),
                                  in_=ofull[s][:, :])


def emit(nc, tc, ctx, io, upoff, stage):
    # 'v2' suffix: DMA host-expanded diagonals instead of gpsimd affine_select,
    # and split the x downcast across Act/DVE
    # 'v3' suffix: keep on-device diagonals, but issue weight + output DMAs from
    # the Activation-engine DGE queue (parallel to x DMAs on the SP queue), and
    # split the x downcast across Act/DVE
    # 'v4' suffix: v3 DMA routing + software-pipelined B2 softmax (lookahead-2
    # logit emission so the in-order PE queue never stalls on the Act exp)
    v2 = stage.endswith('v2')
    v4 = stage.endswith('v4')
    v3 = stage.endswith('v3') or v4
    if v2 or v3:
        stage = stage[:-2]
    if stage == 'noop':
        np_ = ctx.enter_context(tc.tile_pool(name="np", bufs=1))
        t = np_.tile([P, 64], f32, tag="nop", name="nop")
        nc.vector.memset(t, 1.0)
        nc.sync.dma_start(out=io['out'][0, 0:P].rearrange("c h w -> c (h w)")[:, 0:64], in_=t)
        return
    if stage == 'noop2':
        np_ = ctx.enter_context(tc.tile_pool(name="np", bufs=1))
        t = np_.tile([P, 64], f32, tag="nop", name="nop")
        nc.vector.memset(t, 1.0)
        return
    if stage in ('mmbig', 'mmsmall'):
        # equal PE engine-time (bf16, equal total streamed elems), 8x instr count
        mp = ctx.enter_context(tc.tile_pool(name="mp", bufs=1))
        pp = ctx.enter_context(tc.tile_pool(name="pp", bufs=4, space="PSUM"))
        a = mp.tile([P, 1024], bf16, tag="mma", name="mma")
        nc.vector.memset(a, 0.001)
        nmm, mov = (400, 512) if stage == 'mmbig' else (3200, 64)
        acc = None
        for i in range(nmm):
            if i % 4 == 0:
                acc = pp.tile([P, 512], f32, tag="bank", name="bank")
            nc.tensor.matmul(acc[:, 0:mov], lhsT=a[:, (i % 8)*128:(i % 8)*128+128],
                             rhs=a[:, 0:mov], start=(i % 4 == 0), stop=(i % 4 == 3))
        t = mp.tile([P, 64], f32, tag="mmo", name="mmo")
        nc.scalar.copy(out=t, in_=acc[:, 0:64])
        nc.sync.dma_start(out=io['out'][0, 0:P].rearrange("c h w -> c (h w)")[:, 0:64], in_=t)
        return
    if stage in ('selonly', 'conly'):
        return emit_isolate(nc, tc, ctx, io, upoff, stage)
    wp = ctx.enter_context(tc.tile_pool(name="wp", bufs=1))       # weights (kernel lifetime)
    ppL = ctx.enter_context(tc.tile_pool(name="ppL", bufs=1))     # long-lived activations
    # 4-slot PSUM ring (all tiles <= 4KB/partition): lets PE run 2-3 stages
    # ahead of the exp/eviction consumers instead of ping-ponging 2 slots
    ps = ctx.enter_context(tc.tile_pool(name="ps", bufs=4, space="PSUM"))
    sq2 = ctx.enter_context(tc.tile_pool(name="sq2", bufs=2))
    wABstack = ExitStack()
    wAB1 = wABstack.enter_context(tc.tile_pool(name="wAB1", bufs=1))  # B1 weights

    mm = nc.tensor.matmul

    def mmr(out, lhsT, rhs, **kw):
        # float32r: same bits as fp32, 4x faster PE stream for moving dim >= 256
        mm(out, lhsT=lhsT.bitcast(f32r), rhs=rhs.bitcast(f32r), **kw)

    # ---- staged weight loads: phase-A weights first, then x, then the rest,
    # so the critical first-phase DMAs aren't queued behind ~5MB of weights
    wsb = {}
    mmkeys = {'wqkch', 'wbqkch_row', 'wvch', 'outch', 'wqksp', 'wvsp',
              'wbvsp_row', 'outsp'}

    def load_w(keys, wAB):
        for k in keys:
            t = io[k]
            dt = t.dtype
            cast = (lambda a: a.bitcast(f32r)) if (k in mmkeys and dt == f32) \
                else (lambda a: a)
            pool = wAB if k in AB_KEYS else wp
            dma = nc.scalar.dma_start if v3 else nc.sync.dma_start
            if len(t.shape) >= 3:
                tiles = []
                for c in range(t.shape[0]):
                    w = pool.tile(list(t.shape[1:]), dt, tag=f"w_{k}{c}")
                    dma(out=cast(w), in_=cast(t[c]))
                    tiles.append(w)
                wsb[k] = tiles
            else:
                w = pool.tile(list(t.shape), dt, tag=f"w_{k}")
                dma(out=cast(w), in_=cast(t[:, :]))
                wsb[k] = w

    load_w(['bn_b'] if v2 else ['dw_w', 'bn_b'], wAB1)

    # ---- x input tiles: flat [P, NS, Z+4096+64] with zeroed head/tail margins;
    # contiguous DMA (row-strided patterns cost ~4x on the DMA engines). Row
    # under/overflows of the conv window read zeros; only the left image column
    # wraps (corrected with negated-tap matmuls). Issued ahead of the big
    # weight loads so phase A starts immediately.
    Z = 195
    XS = Z + 4096 + 64
    xstack = ExitStack()
    xpool = xstack.enter_context(tc.tile_pool(name="xp", bufs=2))
    xlins = {}

    def mk_xlin(cc):
        # DMA f32 x into a rotating scratch tile, then downcast into the
        # margined bf16 working copy (lives in ppL through phase C: conv
        # input here, residual source in phase C)
        xr = xpool.tile([P, NS, 4096], f32, tag="xraw", name=f"xraw{cc}")
        for s in range(NS):
            nc.sync.dma_start(out=xr[:, s, :].bitcast(f32r),
                              in_=io['x'][s, cc*P:(cc+1)*P].rearrange("c h w -> c (h w)").bitcast(f32r))
        xt = ppL.tile([P, NS, XS], bf16, tag=f"xbfm{cc}", name=f"xbfm{cc}")
        nc.vector.memset(xt[:, :, 0:Z], 0.0)
        nc.vector.memset(xt[:, :, Z+4096:], 0.0)
        if v2 or v3:
            nc.scalar.copy(out=xt[:, 0, Z:Z+4096], in_=xr[:, 0])
            nc.vector.tensor_scalar(out=xt[:, 1, Z:Z+4096], in0=xr[:, 1],
                                    scalar1=1.0, scalar2=None, op0=Alu.mult)
        else:
            nc.scalar.copy(out=xt[:, :, Z:Z+4096], in_=xr)
        xlins[cc] = xt

    mk_xlin(0)
    load_w(['wqkch', 'wbqkch_row', 'wvch', 'wbvch', 'outch', 'outchb'], wAB1)
    mk_xlin(1)

    # ---- constants
    ones_col = wp.tile([P, 1], f32, tag="ones_col", name="ones_col")
    nc.vector.memset(ones_col, 1.0)
    ones_col_bf = wp.tile([P, 1], bf16, tag="ones_col_bf", name="ones_col_bf")
    nc.vector.memset(ones_col_bf, 1.0)
    ones_row = wp.tile([1, P], f32, tag="ones_row", name="ones_row")
    nc.vector.memset(ones_row, 1.0)
    ones32 = wp.tile([P, 32], bf16, tag="ones32", name="ones32")
    nc.vector.memset(ones32, 1.0)
    eps_t = wp.tile([P, 1], f32, tag="eps_t", name="eps_t")
    nc.vector.memset(eps_t, float(EPS))

    # ---- groupnorm stats helper: partials [128,4] x NCH -> (mean, r) each [128, NS] views
    def gn_stats(pool, parts, name):
        acc = ps.tile([1, 4], f32, tag="bank", name="bank")
        for cc in range(NCH):
            mm(acc, lhsT=ones_col, rhs=parts[cc], start=(cc == 0), stop=(cc == NCH-1))
        s4 = pool.tile([1, 4], f32, tag=f"gns4_{name}", name=f"gns4_{name}")
        nc.scalar.copy(out=s4, in_=acc)
        bc = ps.tile([P, 4], f32, tag="bank", name="bank")
        mm(bc, lhsT=ones_row, rhs=s4, start=True, stop=True)
        ex = pool.tile([P, 4], f32, tag=f"gnex_{name}", name=f"gnex_{name}")       # E[x], E[x^2] interleaved
        nc.vector.tensor_scalar(out=ex, in0=bc, scalar1=float(1.0 / CL), scalar2=None,
                                op0=Alu.mult)
        mean = ex[:, 0:4:2]
        msq = pool.tile([P, NS], f32, tag=f"gnmsq_{name}", name=f"gnmsq_{name}")
        nc.vector.tensor_tensor(out=msq, in0=mean, in1=mean, op=Alu.mult)
        var = pool.tile([P, NS], f32, tag=f"gnvar_{name}", name=f"gnvar_{name}")
        nc.vector.tensor_tensor(out=var, in0=ex[:, 1:4:2], in1=msq, op=Alu.subtract)
        std = pool.tile([P, NS], f32, tag=f"gnstd_{name}", name=f"gnstd_{name}")
        nc.scalar.activation(out=std, in_=var, func=Act.Sqrt, bias=eps_t[:, 0:1])
        rbc = pool.tile([P, NS], f32, tag=f"gnr_{name}", name=f"gnr_{name}")
        nc.vector.reciprocal(out=rbc, in_=std)
        return mean, rbc


    with ExitStack() as ctxAB:
        # ============================================================ phase A: dw conv + BN
        # one matmul per tap covering both samples; half-split diagonals rotate
        # in a 2-slot pool so the next build overlaps current matmuls
        xd, p1, xbf = [], [], []
        with ExitStack() as ctxA:
            sA = ctxA.enter_context(tc.tile_pool(name="sA", bufs=2))
            dgd = {}

            def mk_dgd(c):
                t = sA.tile([P, 70, P], bf16, tag="dgd", name=f"dgd{c}")
                nc.sync.dma_start(out=t, in_=io['dwdiag'][c])
                dgd[c] = t

            if v2:
                mk_dgd(0)
            for cc in range(NCH):
                if v2 and cc + 1 < NCH:
                    mk_dgd(cc + 1)
                if cc + 2 < NCH:
                    mk_xlin(cc + 2)
                if v2:
                    dgA = dgB = diagN = None
                else:
                    dgA = sA.tile([P, 25, P], bf16, tag="diag", name=f"dgA{cc}")
                    nc.gpsimd.affine_select(
                        out=dgA, in_=wsb['dw_w'][cc][:, 0:25].unsqueeze(2).broadcast_to([P, 25, P]),
                        compare_op=Alu.is_equal, fill=0.0, base=0,
                        pattern=[[0, 25], [1, P]], channel_multiplier=-1)
                    dgB = sA.tile([P, 24, P], bf16, tag="diag", name=f"dgB{cc}")
                    nc.gpsimd.affine_select(
                        out=dgB, in_=wsb['dw_w'][cc][:, 25:49].unsqueeze(2).broadcast_to([P, 24, P]),
                        compare_op=Alu.is_equal, fill=0.0, base=0,
                        pattern=[[0, 24], [1, P]], channel_multiplier=-1)
                    # negated weights for the 21 left-column wrap-correction taps (kj<3)
                    dwneg = sA.tile([P, 21], bf16, tag="dwneg", name="dwneg", bufs=1)
                    nc.vector.tensor_scalar(out=dwneg.rearrange("p (a b) -> p a b", a=7),
                                            in0=wsb['dw_w'][cc].rearrange("p (a b) -> p a b", a=7)[:, :, 0:3],
                                            scalar1=-1.0, scalar2=None, op0=Alu.mult)
                    diagN = sA.tile([P, 21, P], bf16, tag="diagN", name="diagN", bufs=1)
                    nc.gpsimd.affine_select(
                        out=diagN, in_=dwneg.unsqueeze(2).broadcast_to([P, 21, P]),
                        compare_op=Alu.is_equal, fill=0.0, base=0,
                        pattern=[[0, 21], [1, P]], channel_multiplier=-1)
                xt = xlins[cc]
                xbf.append(xt)
                xds = ppL.tile([P, NS, L], f32, tag=f"xd{cc}", name=f"xd{cc}")
                pc = ppL.tile([P, 4], f32, tag=f"p1_{cc}", name=f"p1_{cc}")
                for sg in [(0, 1)]:
                    acc = ps.tile([P, len(sg), 16, 16], f32, tag="bank", name="bank")
                    xg = xt[:, sg[0]:sg[-1]+1]
                    views = {}
                    for ki in range(7):
                        for kj in range(7):
                            t = ki*7 + kj
                            off = Z + (ki-3)*64 + (kj-3)
                            xv = xg[:, :, off:off+3904].rearrange(
                                "p s (h w) -> p s h w", h=61)[:, :, 0:61:4, 0:61:4]
                            views[t] = xv
                            if v2:
                                dg = dgd[cc][:, t, :]
                            else:
                                dg = dgA[:, t, :] if t < 25 else dgB[:, t-25, :]
                            mm(acc, lhsT=dg, rhs=xv, start=(t == 0), stop=False)
                    for ki in range(7):
                        for kj in range(3):
                            t = ki*7 + kj
                            last = (t == 6*7 + 2)
                            dgn = dgd[cc][:, 49 + ki*3 + kj, :] if v2 \
                                else diagN[:, ki*3 + kj, :]
                            mm(acc[:, :, :, 0:1], lhsT=dgn,
                               rhs=views[t][:, :, :, 0:1], start=False, stop=last)
                    for si, s in enumerate(sg):
                        sq = sq2.tile([P, L], f32, tag="sqscratch", name="sqscratch")
                        nc.vector.tensor_scalar(out=xds[:, s, :], in0=acc[:, si],
                                                scalar1=wsb['bn_b'][cc][:, 0:1], scalar2=None,
                                                op0=Alu.add, op1=Alu.add, accum_out=pc[:, 2*s:2*s+1])
                        nc.scalar.activation(out=sq, in_=xds[:, s, :], func=Act.Square,
                                             accum_out=pc[:, 2*s+1:2*s+2])
                xd.append(xds)
                p1.append(pc)
        xstack.close()
        # B2 + C weights land in a pool carved from the freed x space; their
        # DMAs stream during phase B1's compute window
        wAB2 = wABstack.enter_context(tc.tile_pool(name="wAB2", bufs=1))
        load_w(['wqksp', 'wbqksp', 'wvsp', 'wbvsp_row', 'outsp', 'outspb',
                'updiagh' if v2 else 'upw', 'upb'], wAB2)
        pA = ctxAB.enter_context(tc.tile_pool(name="pA", bufs=1))

        if stage == 'dw':
            for cc in range(NCH):
                for s in range(NS):
                    nc.sync.dma_start(out=io['out'][s, cc*P:(cc+1)*P], in_=xd[cc][:, s, :])
            return

        mean1, r1 = gn_stats(pA, p1, "gn1")
        xn1 = []
        for cc in range(NCH):
            xn = pA.tile([P, NS, L], bf16, tag=f"xn1_{cc}", name=f"xn1_{cc}")
            for s in range(NS):
                nc.vector.tensor_scalar(out=xn[:, s, :], in0=xd[cc][:, s, :],
                                        scalar1=mean1[:, s:s+1], scalar2=r1[:, s:s+1],
                                        op0=Alu.subtract, op1=Alu.mult)
            xn1.append(xn)

        if stage == 'gn1':
            for cc in range(NCH):
                for s in range(NS):
                    nc.sync.dma_start(out=io['out'][s, cc*P:(cc+1)*P], in_=xn1[cc][:, s, :])
            return

        # ============================================================ phase B1: channel attention
        # q^T,k^T [L, 2C] pix-major
        qkT = [pA.tile([P, NS, 2*C], bf16, tag=f"qkT{m}", name=f"qkT{m}") for m in range(2)]
        for s in range(NS):
            for m in range(2):
                for nt in range(2):
                    acc = ps.tile([P, C], f32, tag="bank", name="bank")
                    for cc in range(NCH):
                        mm(acc, lhsT=xn1[cc][:, s, m*P:(m+1)*P],
                           rhs=wsb['wqkch'][cc][:, nt*C:(nt+1)*C], start=(cc == 0), stop=False)
                    mmr(acc, lhsT=ones_row, rhs=wsb['wbqkch_row'][0:1, nt*C:(nt+1)*C],
                       start=False, stop=True)
                    nc.scalar.copy(out=qkT[m][:, s, nt*C:(nt+1)*C], in_=acc)
        # v [C, NS*L] channel-major
        vch = []
        for mv in range(NCH):
            vt = pA.tile([P, NS, L], bf16, tag=f"vch{mv}", name=f"vch{mv}")
            for s in range(NS):
                acc = ps.tile([P, L], f32, tag="bank", name="bank")
                for cc in range(NCH):
                    mm(acc, lhsT=wsb['wvch'][cc][:, mv*P:(mv+1)*P],
                       rhs=xn1[cc][:, s, :], start=(cc == 0), stop=(cc == NCH-1))
                nc.vector.tensor_scalar(out=vt[:, s, :], in0=acc,
                                        scalar1=wsb['wbvch'][mv][:, 0:1], scalar2=None, op0=Alu.add)
            vch.append(vt)
        # S^T = k^T' q^T  [d, c]; exp
        est = [pA.tile([P, NS, C], bf16, tag=f"est{md}", name=f"est{md}") for md in range(NCH)]
        for s in range(NS):
            for md in range(NCH):
                acc = ps.tile([P, C], f32, tag="bank", name="bank")
                for lk in range(2):
                    mm(acc, lhsT=qkT[lk][:, s, C+md*P:C+(md+1)*P], rhs=qkT[lk][:, s, 0:C],
                       start=(lk == 0), stop=(lk == 1))
                nc.scalar.activation(out=est[md][:, s, :], in_=acc, func=Act.Exp)
        # denom (column) + reciprocal
        rd = pA.tile([P, NS, NCH], f32, tag="rdch", name="rdch")
        for s in range(NS):
            for mc in range(NCH):
                acc = ps.tile([P, 1], f32, tag="bank", name="bank")
                for kd in range(NCH):
                    mm(acc, lhsT=est[kd][:, s, mc*P:(mc+1)*P], rhs=ones_col_bf,
                       start=(kd == 0), stop=(kd == NCH-1))
                nc.vector.reciprocal(out=rd[:, s, mc:mc+1], in_=acc)
        # vals = E^T v, normalized by 1/denom per row
        vals = []
        for mc in range(NCH):
            vt = pA.tile([P, NS, L], bf16, tag=f"vals{mc}", name=f"vals{mc}")
            for s in range(NS):
                acc = ps.tile([P, L], f32, tag="bank", name="bank")
                for kd in range(NCH):
                    mm(acc, lhsT=est[kd][:, s, mc*P:(mc+1)*P], rhs=vch[kd][:, s, :],
                       start=(kd == 0), stop=(kd == NCH-1))
                nc.vector.tensor_scalar(out=vt[:, s, :], in0=acc,
                                        scalar1=rd[:, s, mc:mc+1],
                                        scalar2=None, op0=Alu.mult)
            vals.append(vt)
        # x_down2 = x_down + outch @ vals + b ; GN2 partials
        xd2 = [ppL.tile([P, NS, L], f32, tag=f"xd2_{mo}", name=f"xd2_{mo}") for mo in range(NCH)]
        p2 = [ppL.tile([P, 4], f32, tag=f"p2_{mo}", name=f"p2_{mo}") for mo in range(NCH)]
        for mo in range(NCH):
            for s in range(NS):
                acc = ps.tile([P, L], f32, tag="bank", name="bank")
                for cc in range(NCH):
                    mm(acc, lhsT=wsb['outch'][cc][:, mo*P:(mo+1)*P],
                       rhs=vals[cc][:, s, :], start=(cc == 0), stop=(cc == NCH-1))
                sq = sq2.tile([P, L], f32, tag="sqscratch", name="sqscratch")
                nc.vector.scalar_tensor_tensor(out=xd2[mo][:, s, :], in0=acc,
                                               scalar=wsb['outchb'][mo][:, 0:1],
                                               in1=xd[mo][:, s, :], op0=Alu.add, op1=Alu.add,
                                               accum_out=p2[mo][:, 2*s:2*s+1])
                nc.scalar.activation(out=sq, in_=xd2[mo][:, s, :], func=Act.Square,
                                     accum_out=p2[mo][:, 2*s+1:2*s+2])

    if stage == 'ch':
        for cc in range(NCH):
            for s in range(NS):
                nc.sync.dma_start(out=io['out'][s, cc*P:(cc+1)*P], in_=xd2[cc][:, s, :])
        wABstack.close()
        return

    # ============================================================ phase B2: spatial attention
    # up-conv diagonals for cc0/cc1 are built here, while Pool is idle, so
    # phase C's first matmul isn't gated on an affine_select
    sCstack = ExitStack()
    sC = sCstack.enter_context(tc.tile_pool(name="sC", bufs=2))
    updiags = {}

    def mk_updiag(cc):
        # 65 columns: cols 0..63 are the conv taps, col 64 is the identity
        # (residual) tap
        if v2:
            updiags[cc] = wsb['updiagh'][cc]
            return
        ud = sC.tile([P, 65, P], bf16, tag="diag", name=f"updiag{cc}")
        nc.gpsimd.affine_select(
            out=ud, in_=wsb['upw'][cc].unsqueeze(2).broadcast_to([P, 65, P]),
            compare_op=Alu.is_equal, fill=0.0, base=0,
            pattern=[[0, 65], [1, P]], channel_multiplier=-1)
        updiags[cc] = ud

    mk_updiag(0)
    mk_updiag(1)

    with ExitStack() as ctxB:
        pB = ctxB.enter_context(tc.tile_pool(name="pB", bufs=1))
        sB = ctxB.enter_context(tc.tile_pool(name="sB", bufs=3))


        mean2, r2 = gn_stats(pB, p2, "gn2")
        xn2 = []
        for cc in range(NCH):
            xn = pB.tile([P, NS, L], bf16, tag=f"xn2_{cc}", name=f"xn2_{cc}")
            for s in range(NS):
                nc.vector.tensor_scalar(out=xn[:, s, :], in0=xd2[cc][:, s, :],
                                        scalar1=mean2[:, s:s+1], scalar2=r2[:, s:s+1],
                                        op0=Alu.subtract, op1=Alu.mult)
            xn2.append(xn)

        # q,k channel-major [2C, NS*L], bf16 (halves SBUF; logits accumulate f32)
        qksp = []
        for mq in range(2*NCH):
            qt = pB.tile([P, NS, L], bf16, tag=f"qksp{mq}", name=f"qksp{mq}")
            for s in range(NS):
                acc = ps.tile([P, L], f32, tag="bank", name="bank")
                for cc in range(NCH):
                    mm(acc, lhsT=wsb['wqksp'][cc][:, mq*P:(mq+1)*P],
                       rhs=xn2[cc][:, s, :], start=(cc == 0), stop=(cc == NCH-1))
                nc.vector.tensor_scalar(out=qt[:, s, :], in0=acc,
                                        scalar1=wsb['wbqksp'][mq][:, 0:1], scalar2=None, op0=Alu.add)
            qksp.append(qt)
        # v^T pix-major [L, C] per sample, bf16
        vsp = [pB.tile([P, NS, C], bf16, tag=f"vsp{ml}", name=f"vsp{ml}") for ml in range(2)]
        for s in range(NS):
            for ml in range(2):
                acc = ps.tile([P, C], f32, tag="bank", name="bank")
                for cc in range(NCH):
                    mm(acc, lhsT=xn2[cc][:, s, ml*P:(ml+1)*P], rhs=wsb['wvsp'][cc],
                       start=(cc == 0), stop=False)
                mmr(acc, lhsT=ones_row, rhs=wsb['wbvsp_row'], start=False, stop=True)
                nc.scalar.copy(out=vsp[ml][:, s, :], in_=acc)
        # softmax+vals pipelined at 2-head granularity: spps/dvdb alternate the
        # two PSUM slots, so the next pair's logit matmuls overlap this pair's
        # exp/normalize
        valssp = pB.tile([32, HEADS, NS, L], bf16, tag="valssp", name="valssp")
        # x3 output tiles + per-sample projection: emitting the projection right
        # after each sample's softmax keeps PE fed during the other sample's
        # exp/normalize bubbles
        x3p = [ppL.tile([P, NS, 18, 18], bf16, tag=f"x3p{mo}", name=f"x3p{mo}")
               for mo in range(NCH)]
        for mo in range(NCH):
            nc.vector.memset(x3p[mo], 0.0)
        ospw = wsb['outsp'].rearrange("p (h c) -> p h c", h=HEADS)

        def proj_x3(s):
            for mo in range(NCH):
                acc = ps.tile([P, L], f32, tag="bank", name="bank")
                for h in range(HEADS):
                    mm(acc, lhsT=ospw[:, h, mo*P:(mo+1)*P],
                       rhs=valssp[:, h, s, :],
                       start=(h == 0), stop=(h == HEADS-1))
                nc.vector.scalar_tensor_tensor(
                    out=x3p[mo][:, s, 1:17, 1:17],
                    in0=acc.rearrange("p (a b) -> p a b", a=16),
                    scalar=wsb['outspb'][mo][:, 0:1],
                    in1=xd2[mo][:, s, :].rearrange("p (a b) -> p a b", a=16),
                    op0=Alu.add, op1=Alu.add)

        def b2_logits(s, hb, half):
            spps = ps.tile([P, 4, L], f32, tag="bank", name="spps")
            for mk in range(2):
                for hx in range(2):
                    hh = half*2 + hx
                    h = hb*4 + hh
                    po = hh*32
                    mm(spps[:, hx*2+mk, :],
                       lhsT=qksp[NCH + h//4][po:po+32, s, mk*P:(mk+1)*P],
                       rhs=qksp[h//4][po:po+32, s, :],
                       start=True, stop=True, tile_position=(po, 0))
            esp = sB.tile([P, 4, L], bf16, tag="esp", name="esp")
            nc.scalar.activation(out=esp, in_=spps, func=Act.Exp)
            return esp

        def b2_vals(s, hb, half, esp):
            # complete each plane's accumulation group before the next --
            # interleaved groups sharing a PSUM bank lose the first write
            dvdb = ps.tile([32, 4, L], f32, tag="bank", name="dvdb")
            for hx in range(2):
                h = hb*4 + half*2 + hx
                for mk in range(2):
                    mm(dvdb[:, hx, :], lhsT=vsp[mk][:, s, h*32:(h+1)*32],
                       rhs=esp[:, hx*2+mk, :], start=(mk == 0), stop=(mk == 1))
            for hx in range(2):
                for mk in range(2):
                    mm(dvdb[:, 2+hx, :], lhsT=ones32, rhs=esp[:, hx*2+mk, :],
                       start=(mk == 0), stop=(mk == 1))
            rec = sB.tile([32, 2, L], f32, tag="recsp", name="recsp")
            nc.vector.reciprocal(out=rec, in_=dvdb[:, 2:4, :])
            for hx in range(2):
                nc.vector.tensor_tensor(
                    out=valssp[:, hb*4+half*2+hx, s, :],
                    in0=dvdb[:, hx, :], in1=rec[:, hx, :], op=Alu.mult)

        if v4:
            # samples interleaved + lookahead-2: PE always has the next two
            # iterations' logit matmuls queued ahead of the dvdb group that
            # waits on this iteration's exp
            seq = [(s, hb, half) for hb in range(NCH) for half in range(2)
                   for s in range(NS)]
            D = 2
            esp_q = {}
            for i in range(D):
                esp_q[i] = b2_logits(*seq[i])
            for i, it in enumerate(seq):
                if i + D < len(seq):
                    esp_q[i + D] = b2_logits(*seq[i + D])
                b2_vals(*it, esp_q.pop(i))
                if i == len(seq) - 2:
                    # sample 0's heads are all done here; its projection's 36
                    # matmuls keep PE busy through the last iteration's exp
                    proj_x3(0)
            proj_x3(1)
        else:
            for s in range(NS):
                for hb in range(NCH):
                    for half in range(2):
                        b2_vals(s, hb, half, b2_logits(s, hb, half))
                proj_x3(s)

    if stage == 'valssp':
        for h in range(HEADS):
            for s in range(NS):
                nc.sync.dma_start(out=io['out'][s, h*32:(h+1)*32, :],
                                  in_=valssp[:, h, s, :])
        return

    if stage == 'sp':
        for cc in range(NCH):
            for s in range(NS):
                cv = ppL.tile([P, 16, 16], f32, tag="spcv", name="spcv")
                nc.scalar.copy(out=cv, in_=x3p[cc][:, s, 1:17, 1:17])
                nc.sync.dma_start(
                    out=io['out'][s, cc*P:(cc+1)*P].rearrange("p (a b) -> p a b", a=16),
                    in_=cv)
        sCstack.close()
        wABstack.close()
        return

    # ============================================================ phase C: up conv + residual
    # residual added via the identity tap (updiag col 64) against the strided
    # bf16 x view — PE handles strided rhs reads fine, so no ofull prefill.
    # Evictions alternate DVE / Act so the strided writes split across engines.
    with ExitStack() as ctxC:
        sCo = ctxC.enter_context(tc.tile_pool(name="sCo", bufs=3))
        mk_updiag(2)
        for cc in range(NCH):
            updiag = updiags[cc]
            xbf4 = xbf[cc][:, :, Z:Z+4096].rearrange("p s (h w) -> p s h w", h=64)
            ofull = [sCo.tile([P, 4096], f32, tag="ofull", name=f"ofull{cc}_{s}")
                     for s in range(NS)]
            of4 = [o.rearrange("p (a b) -> p a b", a=64) for o in ofull]
            for r1 in range(4):
                for r2 in range(4):
                    ph = r1*4 + r2
                    acc = ps.tile([P, NS, 16, 16], f32, tag="bank", name="upps")
                    for t in range(4):
                        j1, j2 = upoff[ph][t]
                        mm(acc, lhsT=updiag[:, ph*4+t, :],
                           rhs=x3p[cc][:, :, 1+j1:17+j1, 1+j2:17+j2],
                           start=(t == 0), stop=False)
                    mm(acc, lhsT=updiag[:, 64, :],
                       rhs=xbf4[:, :, r1::4, r2::4], start=False, stop=True)
                    for s in range(NS):
                        ov = of4[s][:, r1::4, r2::4]
                        if (ph + s) % 2 == 0:
                            nc.vector.tensor_scalar(
                                out=ov, in0=acc[:, s],
                                scalar1=wsb['upb'][cc][:, 0:1], scalar2=None,
                                op0=Alu.add)
                        else:
                            nc.scalar.add(out=ov, in_=acc[:, s],
                                          add=wsb['upb'][cc][:, 0:1])
            for s in range(NS):
                (nc.scalar.dma_start if v3 else nc.sync.dma_start)(
                    out=io['out'][s, cc*P:(cc+1)*P].rearrange("c h w -> c (h w)"),
                    in_=ofull[s][:, :])
    sCstack.close()
    wABstack.close()


STAGE_OUT_SHAPE = {
    'dw':  [NS, C, L], 'gn1': [NS, C, L], 'ch': [NS, C, L], 'sp': [NS, C, L],
    'valssp': [NS, C, L], 'full': [NS, C, 64, 64],
    'selonly': [NS, C, 64, 64], 'conly': [NS, C, 64, 64], 'noop': [NS, C, 64, 64], 'noop2': [NS, C, 64, 64], 'mmbig': [NS, C, 64, 64], 'mmsmall': [NS, C, 64, 64],
    'fullv2': [NS, C, 64, 64], 'chv2': [NS, C, L], 'spv2': [NS, C, L],
    'fullv3': [NS, C, 64, 64], 'fullv4': [NS, C, 64, 64],
}


def build(d, stage='full', reps=1):
    import ml_dtypes
    nc = bacc.Bacc("TRN2", target_bir_lowering=False, debug=False, num_devices=NCORES)
    io = {}
    io['x'] = nc.declare_dram_parameter("x", [NS, C, 64, 64], f32, isOutput=False)[:]
    for k in DEV_KEYS:
        dt = bf16 if d[k].dtype == ml_dtypes.bfloat16 else f32
        io[k] = nc.declare_dram_parameter(k, list(d[k].shape), dt, isOutput=False)[:]
    io['out'] = nc.declare_dram_parameter("out", STAGE_OUT_SHAPE[stage], f32, isOutput=True)[:]
    with ExitStack() as ctx:
        tc = ctx.enter_context(tile.TileContext(nc))
        for _ in range(reps):
            with ExitStack() as rctx:
                emit(nc, tc, rctx, io, d['upoff'], stage)
    nc.finalize()
    return nc


# ----------------------------------------------------------------------------- runner
class Runner:
    """Builds once; keeps a persistent jitted executable for repeat (timed) runs."""

    def __init__(self, d, stage='full', reps=1):
        self.d = d
        self.stage = stage
        self.nc = build(d, stage, reps)
        self._fn = None

    def _make_fn(self):
        import jax
        from jax.sharding import Mesh, PartitionSpec
        from jax.experimental.shard_map import shard_map
        from concourse.bass2jax import _bass_exec_p, install_neuronx_cc_hook, partition_id_tensor
        install_neuronx_cc_hook()
        nc = self.nc
        part_name = nc.partition_id_tensor.name if nc.partition_id_tensor else None
        in_names, out_names, out_avals, zero_outs = [], [], [], []
        for alloc in nc.m.functions[0].allocations:
            if not isinstance(alloc, mybir.MemoryLocationSet):
                continue
            name = alloc.memorylocations[0].name
            if alloc.kind == "ExternalInput":
                if name != part_name:
                    in_names.append(name)
            elif alloc.kind == "ExternalOutput":
                shape = tuple(alloc.tensor_shape)
                dtype = mybir.dt.np(alloc.dtype)
                out_names.append(name)
                out_avals.append(jax.core.ShapedArray(shape, dtype))
                zero_outs.append(np.zeros(shape, dtype))
        self.in_names = in_names
        self.out_names = out_names
        self.zero_outs = zero_outs
        n_params = len(in_names)
        all_names = in_names + out_names
        if part_name is not None:
            all_names = all_names + [part_name]

        def _body(*args):
            operands = list(args)
            if part_name is not None:
                operands.append(partition_id_tensor())
            outs = _bass_exec_p.bind(
                *operands, out_avals=tuple(out_avals), in_names=tuple(all_names),
                out_names=tuple(out_names), lowering_input_output_aliases=(),
                sim_require_finite=False, sim_require_nnan=False, nc=nc)
            return tuple(outs)

        devices = jax.devices()[:NCORES]
        mesh = Mesh(np.asarray(devices), ("core",))
        self._mesh = mesh
        donate = tuple(range(n_params, n_params + len(out_names)))
        self._fn = jax.jit(
            shard_map(_body, mesh=mesh,
                      in_specs=(PartitionSpec("core"),) * (n_params + len(out_names)),
                      out_specs=(PartitionSpec("core"),) * len(out_names)),
            donate_argnums=donate, keep_unused=True)
        self._out_shapes = [tuple(a.shape) for a in out_avals]

    def run(self, in_maps, timeit=False, reps=5):
        import jax, time
        from jax.sharding import NamedSharding, PartitionSpec
        if self._fn is None:
            self._make_fn()
        concat_in = [np.concatenate([np.asarray(m[name]) for m in in_maps], axis=0)
                     for name in self.in_names]
        concat_zero = [np.zeros((NCORES * z.shape[0], *z.shape[1:]), z.dtype)
                       for z in self.zero_outs]
        out = self._fn(*concat_in, *concat_zero)
        jax.block_until_ready(out)
        tmin = None
        if timeit:
            sh = NamedSharding(self._mesh, PartitionSpec("core"))
            dev_in = [jax.device_put(a, sh) for a in concat_in]
            jax.block_until_ready(dev_in)
            times = []
            for _ in range(reps):
                cz = [jax.device_put(np.zeros_like(z), sh) for z in concat_zero]
                jax.block_until_ready(cz)
                t0 = time.perf_counter()
                o2 = self._fn(*dev_in, *cz)
                jax.block_until_ready(o2)
                times.append(time.perf_counter() - t0)
            tmin = min(times)
            import os
            if os.environ.get('PRINT_TIMES'):
                print('times_ms:', [round(t*1e3,2) for t in times])
        outs = [np.asarray(o).reshape(NCORES, *self._out_shapes[i])
                for i, o in enumerate(out)]
        per_core = [{name: outs[i][c] for i, name in enumerate(self.out_names)}
                    for c in range(NCORES)]
        return per_core, tmin


_RUNNER_CACHE = {}


def _get_runner(d, stage='full'):
    if stage not in _RUNNER_CACHE:
        _RUNNER_CACHE[stage] = Runner(d, stage)
    return _RUNNER_CACHE[stage]


def make_in_maps(d, x):
    """x: [16, C, 64, 64] -> per-core input maps."""
    maps = []
    for c in range(NCORES):
        m = {k: d[k] for k in DEV_KEYS}
        m['x'] = np.ascontiguousarray(x[c*NS:(c+1)*NS])
        maps.append(m)
    return maps


def kernel(**inputs):
    inputs = {k: np.asarray(v) for k, v in inputs.items()}
    d = host_precompute(inputs)
    r = _get_runner(d, 'fullv4')
    res, _ = r.run(make_in_maps(d, inputs['x']))
    return np.concatenate([res[c]['out'] for c in range(NCORES)], axis=0)



# revision 34
# speedup vs baseline: 1.0275x; 1.0275x over previous
"""Trainium2 Bass kernel for nn_ConvCombAttention (dense_transformer).

Sharding: pure data parallel — 16 samples split as 2 per NeuronCore across 8 cores.
All weights are host-folded (BN into depthwise conv, GroupNorm affine + softmax
scale into the 1x1 projections) and replicated.

Device algorithm per core (2 samples; bf16 GEMM datapath, f32 accumulation and
statistics):
  - depthwise 7x7/s4 conv as 49 PSUM-accumulated bf16 matmuls with diagonal lhsT
    (diagonals built on-device by gpsimd.affine_select from compact weights);
    x is downcast once to a margined bf16 working copy that also serves as the
    phase-C residual source
  - GroupNorm(1 group) via matmul cross-partition reductions + per-partition affine
  - channel attention computed entirely in S^T layout (no transposes anywhere);
    softmax denominators via ones-matmuls, normalization folded into PSUM evictions
  - spatial MHA with K=32 row-tiled matmuls and 32-wide col-tiled vals/denom matmuls
  - transposed depthwise conv (8x8/s4) as 64 diag bf16 matmuls + an identity
    65th tap that adds the x residual inside the PSUM accumulation; evictions
    alternate DVE/Act so the strided interleave writes split across engines

Active variant ('fullv4'): weight + output DMAs issue from the Activation-engine
DGE queue (parallel to x input DMAs on SP), the x downcast splits across
Act/DVE, and the spatial-softmax loop is software-pipelined with lookahead 2
(samples interleaved) so the in-order PE queue never stalls on the Act exp.
"""
import sys, os
for _p in ('/opt/trn_rl_repo', '/root/.axon_site/_ro/trn_rl_repo'):
    if os.path.isdir(_p) and _p not in sys.path:
        sys.path.append(_p)

import numpy as np
import concourse.bass as bass
import concourse.mybir as mybir
import concourse.tile as tile
from concourse import bacc
from contextlib import ExitStack

f32 = mybir.dt.float32
f32r = mybir.dt.float32r
bf16 = mybir.dt.bfloat16
Alu = mybir.AluOpType
Act = mybir.ActivationFunctionType

C = 384; HEADS = 12; HD = 32
EPS = 1e-5
H = W = 16; L = H * W          # downsampled spatial
NS = 2                          # samples per core
NCORES = 8
P = 128; NCH = C // P           # channel chunks
CUT = None                      # debug: 'nodma' | 'noevict' | 'nomm'
CL = np.float32(C * L)          # groupnorm normalizer


# ----------------------------------------------------------------------------- host fold
def host_precompute(inp):
    d = {}
    import ml_dtypes
    s = (inp['bn_gamma'] / np.sqrt(inp['bn_var'] + EPS)).astype(np.float32)
    dw = (inp['dw_w'].reshape(C, 49) * s[:, None]).astype(np.float32)
    # bf16: the whole depthwise-conv path (diagonals, x) runs in bf16 (PE 2x)
    d['dw_w'] = dw.reshape(NCH, P, 49).astype(ml_dtypes.bfloat16).copy()
    d['bn_b'] = (inp['bn_beta'] - inp['bn_mean'] * s).astype(np.float32).reshape(NCH, P, 1).copy()

    Wg1 = (inp['pwise_ch_w'] * inp['lnch_gamma'][None, :]).astype(np.float32)
    Wb1 = (inp['pwise_ch_w'] @ inp['lnch_beta']).astype(np.float32)
    sc1 = np.float32(1.0 / np.sqrt(np.sqrt(np.float32(L))))
    d['wqkch'] = np.concatenate([Wg1[0:C] * sc1, Wg1[C:2*C] * sc1], 0).T.reshape(
        NCH, P, 2*C).astype(ml_dtypes.bfloat16).copy()
    d['wbqkch_row'] = (np.concatenate([Wb1[0:C], Wb1[C:2*C]]) * sc1)[None, :].astype(np.float32).copy()
    d['wvch'] = Wg1[2*C:3*C].T.copy().reshape(NCH, P, C).astype(ml_dtypes.bfloat16).copy()
    d['wbvch'] = Wb1[2*C:3*C].astype(np.float32).reshape(NCH, P, 1).copy()
    d['outch'] = inp['outch_w'].T.copy().reshape(NCH, P, C).astype(ml_dtypes.bfloat16).copy()
    d['outchb'] = inp['outch_b'].astype(np.float32).reshape(NCH, P, 1).copy()

    Wg2 = (inp['pwise_sp_w'] * inp['lnsp_gamma'][None, :]).astype(np.float32)
    Wb2 = (inp['pwise_sp_w'] @ inp['lnsp_beta']).astype(np.float32)
    sc2 = np.float32(1.0 / np.sqrt(np.sqrt(np.float32(HD))))
    qrows = np.concatenate([np.arange(h*3*HD, h*3*HD+HD) for h in range(HEADS)])
    krows = qrows + HD
    vrows = qrows + 2*HD
    d['wqksp'] = np.concatenate([Wg2[qrows] * sc2, Wg2[krows] * sc2], 0).T.reshape(
        NCH, P, 2*C).astype(ml_dtypes.bfloat16).copy()
    d['wbqksp'] = (np.concatenate([Wb2[qrows], Wb2[krows]]) * sc2).astype(np.float32).reshape(2*NCH, P, 1).copy()
    d['wvsp'] = Wg2[vrows].T.copy().reshape(NCH, P, C).astype(ml_dtypes.bfloat16).copy()
    d['wbvsp_row'] = Wb2[vrows][None, :].astype(np.float32).copy()
    # [32(d), 12(h)*384(oc)]: per-head K=32 blocks, partition base 0; bf16 (SBUF)
    import ml_dtypes
    d['outsp'] = inp['outsp_w'].T.reshape(HEADS, HD, C).transpose(1, 0, 2).reshape(
        HD, HEADS*C).astype(ml_dtypes.bfloat16).copy()
    d['outspb'] = inp['outsp_b'].astype(np.float32).reshape(NCH, P, 1).copy()

    # transposed depthwise conv: out[c,4a+r1,4b+r2] = sum_t K[c,(r1,r2),t] x3p[c,a+j1,b+j2]
    # stored bf16: the up-conv path (updiag, x3p) runs in bf16
    wf = inp['up_w'][:, 0, ::-1, ::-1].astype(np.float32)
    taps = {r: [(j, 5 - r + 4*j) for j in (-1, 0, 1) if 0 <= 5 - r + 4*j < 8] for r in range(4)}
    # col 64 = 1.0: identity tap, used to add the x residual inside the PSUM
    # accumulation (kills the ofull prefill copies)
    upw = np.zeros((C, 65), np.float32)
    upw[:, 64] = 1.0
    upoff = []
    for r1 in range(4):
        for r2 in range(4):
            lst = []
            t = 0
            for (j1, m1) in taps[r1]:
                for (j2, m2) in taps[r2]:
                    upw[:, (r1*4+r2)*4 + t] = wf[:, m1, m2]
                    lst.append((j1, j2))
                    t += 1
            upoff.append(lst)
    d['upw'] = upw.reshape(NCH, P, 65).astype(ml_dtypes.bfloat16).copy()
    d['upoff'] = upoff    # python-side constant (16 x 4 (j1,j2))
    d['upb'] = inp['up_b'].astype(np.float32).reshape(NCH, P, 1).copy()

    # v2 variant: host-expanded diagonal weight tensors (DMA'd instead of
    # built on-device with gpsimd.affine_select). Layout [NCH, P, T, P] with
    # diag[c, p, t, q] = w[c, p, t] * (p == q).
    dwf = np.asarray(d['dw_w'], np.float32)            # [NCH, P, 49]
    dd = np.zeros((NCH, P, 70, P), np.float32)
    eye = np.eye(P, dtype=np.float32)
    for cc in range(NCH):
        dd[cc, :, 0:49, :] = dwf[cc][:, :, None] * eye[:, None, :]
        neg = -dwf[cc].reshape(P, 7, 7)[:, :, 0:3].reshape(P, 21)
        dd[cc, :, 49:70, :] = neg[:, :, None] * eye[:, None, :]
    d['dwdiag'] = dd.astype(ml_dtypes.bfloat16).copy()
    upf = np.asarray(d['upw'], np.float32)             # [NCH, P, 65]
    ud = upf[:, :, :, None] * eye[None, :, None, :]
    d['updiagh'] = ud.astype(ml_dtypes.bfloat16).copy()
    return d


DEV_KEYS = ['dw_w', 'bn_b', 'wqkch', 'wbqkch_row', 'wvch', 'wbvch', 'outch', 'outchb',
            'wqksp', 'wbqksp', 'wvsp', 'wbvsp_row', 'outsp', 'outspb', 'upw', 'upb',
            'dwdiag', 'updiagh']


# ----------------------------------------------------------------------------- device program
AB_KEYS = {'wqkch', 'wbqkch_row', 'wvch', 'wbvch', 'outch', 'outchb',
           'wqksp', 'wbqksp', 'wvsp', 'wbvsp_row', 'outsp', 'outspb', 'updiagh'}


def emit_isolate(nc, tc, ctx, io, upoff, stage):
    """Timing-isolation stages: run one suspect section on dummy data."""
    wp = ctx.enter_context(tc.tile_pool(name="wp", bufs=1))
    ps = ctx.enter_context(tc.tile_pool(name="ps", bufs=4, space="PSUM"))
    mm = nc.tensor.matmul
    upwt = []
    for c in range(NCH):
        w = wp.tile([P, 65], bf16, tag=f"upw{c}", name=f"upw{c}")
        nc.sync.dma_start(out=w, in_=io['upw'][c])
        upwt.append(w)
    upbt = []
    for c in range(NCH):
        w = wp.tile([P, 1], f32, tag=f"upb{c}", name=f"upb{c}")
        nc.sync.dma_start(out=w, in_=io['upb'][c])
        upbt.append(w)
    with ExitStack() as c1:
        pp = c1.enter_context(tc.tile_pool(name="pp", bufs=1))
        sC = c1.enter_context(tc.tile_pool(name="sC", bufs=2))
        sCo = c1.enter_context(tc.tile_pool(name="sCo", bufs=3))
        xbf = [pp.tile([P, NS, 4096], bf16, tag=f"xbf{c}", name=f"xbf{c}") for c in range(NCH)]
        x3p = [pp.tile([P, NS, 18, 18], bf16, tag=f"x3p{c}", name=f"x3p{c}") for c in range(NCH)]
        for c in range(NCH):
            nc.vector.memset(xbf[c], 0.5)
            nc.vector.memset(x3p[c], 0.5)
        if stage == 'selonly':
            # 3 bf16 affine_selects only
            for cc in range(NCH):
                ud = sC.tile([P, 65, P], bf16, tag="diag", name=f"ud{cc}")
                nc.gpsimd.affine_select(
                    out=ud, in_=upwt[cc].unsqueeze(2).broadcast_to([P, 65, P]),
                    compare_op=Alu.is_equal, fill=0.0, base=0,
                    pattern=[[0, 65], [1, P]], channel_multiplier=-1)
                sc = sC.tile([P, 64], f32, tag="sc", name=f"sc{cc}")
                nc.scalar.copy(out=sc, in_=ud[:, 0:64, 0])
                nc.sync.dma_start(out=io['out'][0, cc*P:(cc+1)*P].rearrange(
                    "c h w -> c (h w)")[:, 0:64], in_=sc)
            return
        # 'conly': full phase C structure on dummy data
        for cc in range(NCH):
            ud = sC.tile([P, 65, P], bf16, tag="diag", name=f"ud{cc}")
            nc.gpsimd.affine_select(
                out=ud, in_=upwt[cc].unsqueeze(2).broadcast_to([P, 65, P]),
                compare_op=Alu.is_equal, fill=0.0, base=0,
                pattern=[[0, 65], [1, P]], channel_multiplier=-1)
            xbf4 = xbf[cc].rearrange("p s (h w) -> p s h w", h=64)
            ofull = [sCo.tile([P, 4096], f32, tag="ofull", name=f"of{cc}_{s}")
                     for s in range(NS)]
            of4 = [o.rearrange("p (a b) -> p a b", a=64) for o in ofull]
            for r1 in range(4):
                for r2 in range(4):
                    ph = r1*4 + r2
                    acc = ps.tile([P, NS, 16, 16], f32, tag="bank", name="upps")
                    for t in range(4):
                        j1, j2 = upoff[ph][t]
                        mm(acc, lhsT=ud[:, ph*4+t, :],
                           rhs=x3p[cc][:, :, 1+j1:17+j1, 1+j2:17+j2],
                           start=(t == 0), stop=(t == 3))
                    for s in range(NS):
                        nc.vector.scalar_tensor_tensor(
                            out=of4[s][:, r1::4, r2::4], in0=acc[:, s],
                            scalar=upbt[cc][:, 0:1],
                            in1=xbf4[:, s, r1::4, r2::4],
                            op0=Alu.add, op1=Alu.add)
            for s in range(NS):
                nc.sync.dma_start(out=io['out'][s, cc*P:(cc+1)*P].rearrange("c h w -> c (h w)"),
                                  in_=ofull[s][:, :])


def emit(nc, tc, ctx, io, upoff, stage):
    # 'v2' suffix: DMA host-expanded diagonals instead of gpsimd affine_select,
    # and split the x downcast across Act/DVE
    # 'v3' suffix: keep on-device diagonals, but issue weight + output DMAs from
    # the Activation-engine DGE queue (parallel to x DMAs on the SP queue), and
    # split the x downcast across Act/DVE
    # 'v4' suffix: v3 DMA routing + software-pipelined B2 softmax (lookahead-2
    # logit emission so the in-order PE queue never stalls on the Act exp)
    v2 = stage.endswith('v2')
    v4 = stage.endswith('v4')
    v3 = stage.endswith('v3') or v4
    if v2 or v3:
        stage = stage[:-2]
    if stage == 'noop':
        np_ = ctx.enter_context(tc.tile_pool(name="np", bufs=1))
        t = np_.tile([P, 64], f32, tag="nop", name="nop")
        nc.vector.memset(t, 1.0)
        nc.sync.dma_start(out=io['out'][0, 0:P].rearrange("c h w -> c (h w)")[:, 0:64], in_=t)
        return
    if stage == 'noop2':
        np_ = ctx.enter_context(tc.tile_pool(name="np", bufs=1))
        t = np_.tile([P, 64], f32, tag="nop", name="nop")
        nc.vector.memset(t, 1.0)
        return
    if stage in ('mmbig', 'mmsmall'):
        # equal PE engine-time (bf16, equal total streamed elems), 8x instr count
        mp = ctx.enter_context(tc.tile_pool(name="mp", bufs=1))
        pp = ctx.enter_context(tc.tile_pool(name="pp", bufs=4, space="PSUM"))
        a = mp.tile([P, 1024], bf16, tag="mma", name="mma")
        nc.vector.memset(a, 0.001)
        nmm, mov = (400, 512) if stage == 'mmbig' else (3200, 64)
        acc = None
        for i in range(nmm):
            if i % 4 == 0:
                acc = pp.tile([P, 512], f32, tag="bank", name="bank")
            nc.tensor.matmul(acc[:, 0:mov], lhsT=a[:, (i % 8)*128:(i % 8)*128+128],
                             rhs=a[:, 0:mov], start=(i % 4 == 0), stop=(i % 4 == 3))
        t = mp.tile([P, 64], f32, tag="mmo", name="mmo")
        nc.scalar.copy(out=t, in_=acc[:, 0:64])
        nc.sync.dma_start(out=io['out'][0, 0:P].rearrange("c h w -> c (h w)")[:, 0:64], in_=t)
        return
    if stage in ('selonly', 'conly'):
        return emit_isolate(nc, tc, ctx, io, upoff, stage)
    wp = ctx.enter_context(tc.tile_pool(name="wp", bufs=1))       # weights (kernel lifetime)
    ppL = ctx.enter_context(tc.tile_pool(name="ppL", bufs=1))     # long-lived activations
    # 4-slot PSUM ring (all tiles <= 4KB/partition): lets PE run 2-3 stages
    # ahead of the exp/eviction consumers instead of ping-ponging 2 slots
    ps = ctx.enter_context(tc.tile_pool(name="ps", bufs=4, space="PSUM"))
    sq2 = ctx.enter_context(tc.tile_pool(name="sq2", bufs=2))
    wABstack = ExitStack()
    wAB1 = wABstack.enter_context(tc.tile_pool(name="wAB1", bufs=1))  # B1 weights

    mm = nc.tensor.matmul

    def mmr(out, lhsT, rhs, **kw):
        # float32r: same bits as fp32, 4x faster PE stream for moving dim >= 256
        mm(out, lhsT=lhsT.bitcast(f32r), rhs=rhs.bitcast(f32r), **kw)

    # ---- staged weight loads: phase-A weights first, then x, then the rest,
    # so the critical first-phase DMAs aren't queued behind ~5MB of weights
    wsb = {}
    mmkeys = {'wqkch', 'wbqkch_row', 'wvch', 'outch', 'wqksp', 'wvsp',
              'wbvsp_row', 'outsp'}

    def load_w(keys, wAB):
        for k in keys:
            t = io[k]
            dt = t.dtype
            cast = (lambda a: a.bitcast(f32r)) if (k in mmkeys and dt == f32) \
                else (lambda a: a)
            pool = wAB if k in AB_KEYS else wp
            dma = nc.scalar.dma_start if v3 else nc.sync.dma_start
            if len(t.shape) >= 3:
                tiles = []
                for c in range(t.shape[0]):
                    w = pool.tile(list(t.shape[1:]), dt, tag=f"w_{k}{c}")
                    dma(out=cast(w), in_=cast(t[c]))
                    tiles.append(w)
                wsb[k] = tiles
            else:
                w = pool.tile(list(t.shape), dt, tag=f"w_{k}")
                dma(out=cast(w), in_=cast(t[:, :]))
                wsb[k] = w

    load_w(['bn_b'] if v2 else ['dw_w', 'bn_b'], wAB1)

    # ---- x input tiles: flat [P, NS, Z+4096+64] with zeroed head/tail margins;
    # contiguous DMA (row-strided patterns cost ~4x on the DMA engines). Row
    # under/overflows of the conv window read zeros; only the left image column
    # wraps (corrected with negated-tap matmuls). Issued ahead of the big
    # weight loads so phase A starts immediately.
    Z = 195
    XS = Z + 4096 + 64
    xstack = ExitStack()
    xpool = xstack.enter_context(tc.tile_pool(name="xp", bufs=2))
    xlins = {}

    def mk_xlin(cc):
        # DMA f32 x into a rotating scratch tile, then downcast into the
        # margined bf16 working copy (lives in ppL through phase C: conv
        # input here, residual source in phase C)
        xr = xpool.tile([P, NS, 4096], f32, tag="xraw", name=f"xraw{cc}")
        for s in range(NS):
            nc.sync.dma_start(out=xr[:, s, :].bitcast(f32r),
                              in_=io['x'][s, cc*P:(cc+1)*P].rearrange("c h w -> c (h w)").bitcast(f32r))
        xt = ppL.tile([P, NS, XS], bf16, tag=f"xbfm{cc}", name=f"xbfm{cc}")
        nc.vector.memset(xt[:, :, 0:Z], 0.0)
        nc.vector.memset(xt[:, :, Z+4096:], 0.0)
        if v2 or v3:
            nc.scalar.copy(out=xt[:, 0, Z:Z+4096], in_=xr[:, 0])
            nc.vector.tensor_scalar(out=xt[:, 1, Z:Z+4096], in0=xr[:, 1],
                                    scalar1=1.0, scalar2=None, op0=Alu.mult)
        else:
            nc.scalar.copy(out=xt[:, :, Z:Z+4096], in_=xr)
        xlins[cc] = xt

    mk_xlin(0)
    load_w(['wqkch', 'wbqkch_row', 'wvch', 'wbvch', 'outch', 'outchb'], wAB1)
    mk_xlin(1)

    # ---- constants
    ones_col = wp.tile([P, 1], f32, tag="ones_col", name="ones_col")
    nc.vector.memset(ones_col, 1.0)
    ones_col_bf = wp.tile([P, 1], bf16, tag="ones_col_bf", name="ones_col_bf")
    nc.vector.memset(ones_col_bf, 1.0)
    ones_row = wp.tile([1, P], f32, tag="ones_row", name="ones_row")
    nc.vector.memset(ones_row, 1.0)
    ones32 = wp.tile([P, 32], bf16, tag="ones32", name="ones32")
    nc.vector.memset(ones32, 1.0)
    eps_t = wp.tile([P, 1], f32, tag="eps_t", name="eps_t")
    nc.vector.memset(eps_t, float(EPS))

    # ---- groupnorm stats helper: partials [128,4] x NCH -> (mean, r) each [128, NS] views
    def gn_stats(pool, parts, name):
        acc = ps.tile([1, 4], f32, tag="bank", name="bank")
        for cc in range(NCH):
            mm(acc, lhsT=ones_col, rhs=parts[cc], start=(cc == 0), stop=(cc == NCH-1))
        s4 = pool.tile([1, 4], f32, tag=f"gns4_{name}", name=f"gns4_{name}")
        nc.scalar.copy(out=s4, in_=acc)
        bc = ps.tile([P, 4], f32, tag="bank", name="bank")
        mm(bc, lhsT=ones_row, rhs=s4, start=True, stop=True)
        ex = pool.tile([P, 4], f32, tag=f"gnex_{name}", name=f"gnex_{name}")       # E[x], E[x^2] interleaved
        nc.vector.tensor_scalar(out=ex, in0=bc, scalar1=float(1.0 / CL), scalar2=None,
                                op0=Alu.mult)
        mean = ex[:, 0:4:2]
        msq = pool.tile([P, NS], f32, tag=f"gnmsq_{name}", name=f"gnmsq_{name}")
        nc.vector.tensor_tensor(out=msq, in0=mean, in1=mean, op=Alu.mult)
        var = pool.tile([P, NS], f32, tag=f"gnvar_{name}", name=f"gnvar_{name}")
        nc.vector.tensor_tensor(out=var, in0=ex[:, 1:4:2], in1=msq, op=Alu.subtract)
        std = pool.tile([P, NS], f32, tag=f"gnstd_{name}", name=f"gnstd_{name}")
        nc.scalar.activation(out=std, in_=var, func=Act.Sqrt, bias=eps_t[:, 0:1])
        rbc = pool.tile([P, NS], f32, tag=f"gnr_{name}", name=f"gnr_{name}")
        nc.vector.reciprocal(out=rbc, in_=std)
        return mean, rbc


    with ExitStack() as ctxAB:
        # ============================================================ phase A: dw conv + BN
        # one matmul per tap covering both samples; half-split diagonals rotate
        # in a 2-slot pool so the next build overlaps current matmuls
        xd, p1, xbf = [], [], []
        with ExitStack() as ctxA:
            sA = ctxA.enter_context(tc.tile_pool(name="sA", bufs=2))
            dgd = {}

            def mk_dgd(c):
                t = sA.tile([P, 70, P], bf16, tag="dgd", name=f"dgd{c}")
                nc.sync.dma_start(out=t, in_=io['dwdiag'][c])
                dgd[c] = t

            if v2:
                mk_dgd(0)
            for cc in range(NCH):
                if v2 and cc + 1 < NCH:
                    mk_dgd(cc + 1)
                if cc + 2 < NCH:
                    mk_xlin(cc + 2)
                if v2:
                    dgA = dgB = diagN = None
                else:
                    dgA = sA.tile([P, 25, P], bf16, tag="diag", name=f"dgA{cc}")
                    nc.gpsimd.affine_select(
                        out=dgA, in_=wsb['dw_w'][cc][:, 0:25].unsqueeze(2).broadcast_to([P, 25, P]),
                        compare_op=Alu.is_equal, fill=0.0, base=0,
                        pattern=[[0, 25], [1, P]], channel_multiplier=-1)
                    dgB = sA.tile([P, 24, P], bf16, tag="diag", name=f"dgB{cc}")
                    nc.gpsimd.affine_select(
                        out=dgB, in_=wsb['dw_w'][cc][:, 25:49].unsqueeze(2).broadcast_to([P, 24, P]),
                        compare_op=Alu.is_equal, fill=0.0, base=0,
                        pattern=[[0, 24], [1, P]], channel_multiplier=-1)
                    # negated weights for the 21 left-column wrap-correction taps (kj<3)
                    dwneg = sA.tile([P, 21], bf16, tag="dwneg", name="dwneg", bufs=1)
                    nc.vector.tensor_scalar(out=dwneg.rearrange("p (a b) -> p a b", a=7),
                                            in0=wsb['dw_w'][cc].rearrange("p (a b) -> p a b", a=7)[:, :, 0:3],
                                            scalar1=-1.0, scalar2=None, op0=Alu.mult)
                    diagN = sA.tile([P, 21, P], bf16, tag="diagN", name="diagN", bufs=1)
                    nc.gpsimd.affine_select(
                        out=diagN, in_=dwneg.unsqueeze(2).broadcast_to([P, 21, P]),
                        compare_op=Alu.is_equal, fill=0.0, base=0,
                        pattern=[[0, 21], [1, P]], channel_multiplier=-1)
                xt = xlins[cc]
                xbf.append(xt)
                xds = ppL.tile([P, NS, L], f32, tag=f"xd{cc}", name=f"xd{cc}")
                pc = ppL.tile([P, 4], f32, tag=f"p1_{cc}", name=f"p1_{cc}")
                for sg in [(0, 1)]:
                    acc = ps.tile([P, len(sg), 16, 16], f32, tag="bank", name="bank")
                    xg = xt[:, sg[0]:sg[-1]+1]
                    views = {}
                    for ki in range(7):
                        for kj in range(7):
                            t = ki*7 + kj
                            off = Z + (ki-3)*64 + (kj-3)
                            xv = xg[:, :, off:off+3904].rearrange(
                                "p s (h w) -> p s h w", h=61)[:, :, 0:61:4, 0:61:4]
                            views[t] = xv
                            if v2:
                                dg = dgd[cc][:, t, :]
                            else:
                                dg = dgA[:, t, :] if t < 25 else dgB[:, t-25, :]
                            mm(acc, lhsT=dg, rhs=xv, start=(t == 0), stop=False)
                    for ki in range(7):
                        for kj in range(3):
                            t = ki*7 + kj
                            last = (t == 6*7 + 2)
                            dgn = dgd[cc][:, 49 + ki*3 + kj, :] if v2 \
                                else diagN[:, ki*3 + kj, :]
                            mm(acc[:, :, :, 0:1], lhsT=dgn,
                               rhs=views[t][:, :, :, 0:1], start=False, stop=last)
                    for si, s in enumerate(sg):
                        sq = sq2.tile([P, L], f32, tag="sqscratch", name="sqscratch")
                        nc.vector.tensor_scalar(out=xds[:, s, :], in0=acc[:, si],
                                                scalar1=wsb['bn_b'][cc][:, 0:1], scalar2=None,
                                                op0=Alu.add, op1=Alu.add, accum_out=pc[:, 2*s:2*s+1])
                        nc.scalar.activation(out=sq, in_=xds[:, s, :], func=Act.Square,
                                             accum_out=pc[:, 2*s+1:2*s+2])
                xd.append(xds)
                p1.append(pc)
        xstack.close()
        # B2 + C weights land in a pool carved from the freed x space; their
        # DMAs stream during phase B1's compute window
        wAB2 = wABstack.enter_context(tc.tile_pool(name="wAB2", bufs=1))
        load_w(['wqksp', 'wbqksp', 'wvsp', 'wbvsp_row', 'outsp', 'outspb',
                'updiagh' if v2 else 'upw', 'upb'], wAB2)
        pA = ctxAB.enter_context(tc.tile_pool(name="pA", bufs=1))

        if stage == 'dw':
            for cc in range(NCH):
                for s in range(NS):
                    nc.sync.dma_start(out=io['out'][s, cc*P:(cc+1)*P], in_=xd[cc][:, s, :])
            return

        mean1, r1 = gn_stats(pA, p1, "gn1")
        xn1 = []
        for cc in range(NCH):
            xn = pA.tile([P, NS, L], bf16, tag=f"xn1_{cc}", name=f"xn1_{cc}")
            for s in range(NS):
                nc.vector.tensor_scalar(out=xn[:, s, :], in0=xd[cc][:, s, :],
                                        scalar1=mean1[:, s:s+1], scalar2=r1[:, s:s+1],
                                        op0=Alu.subtract, op1=Alu.mult)
            xn1.append(xn)

        if stage == 'gn1':
            for cc in range(NCH):
                for s in range(NS):
                    nc.sync.dma_start(out=io['out'][s, cc*P:(cc+1)*P], in_=xn1[cc][:, s, :])
            return

        # ============================================================ phase B1: channel attention
        # q^T,k^T [L, 2C] pix-major
        qkT = [pA.tile([P, NS, 2*C], bf16, tag=f"qkT{m}", name=f"qkT{m}") for m in range(2)]
        for s in range(NS):
            for m in range(2):
                for nt in range(2):
                    acc = ps.tile([P, C], f32, tag="bank", name="bank")
                    for cc in range(NCH):
                        mm(acc, lhsT=xn1[cc][:, s, m*P:(m+1)*P],
                           rhs=wsb['wqkch'][cc][:, nt*C:(nt+1)*C], start=(cc == 0), stop=False)
                    mmr(acc, lhsT=ones_row, rhs=wsb['wbqkch_row'][0:1, nt*C:(nt+1)*C],
                       start=False, stop=True)
                    nc.scalar.copy(out=qkT[m][:, s, nt*C:(nt+1)*C], in_=acc)
        # v [C, NS*L] channel-major
        vch = []
        for mv in range(NCH):
            vt = pA.tile([P, NS, L], bf16, tag=f"vch{mv}", name=f"vch{mv}")
            for s in range(NS):
                acc = ps.tile([P, L], f32, tag="bank", name="bank")
                for cc in range(NCH):
                    mm(acc, lhsT=wsb['wvch'][cc][:, mv*P:(mv+1)*P],
                       rhs=xn1[cc][:, s, :], start=(cc == 0), stop=(cc == NCH-1))
                nc.vector.tensor_scalar(out=vt[:, s, :], in0=acc,
                                        scalar1=wsb['wbvch'][mv][:, 0:1], scalar2=None, op0=Alu.add)
            vch.append(vt)
        # S^T = k^T' q^T  [d, c]; exp
        est = [pA.tile([P, NS, C], bf16, tag=f"est{md}", name=f"est{md}") for md in range(NCH)]
        for s in range(NS):
            for md in range(NCH):
                acc = ps.tile([P, C], f32, tag="bank", name="bank")
                for lk in range(2):
                    mm(acc, lhsT=qkT[lk][:, s, C+md*P:C+(md+1)*P], rhs=qkT[lk][:, s, 0:C],
                       start=(lk == 0), stop=(lk == 1))
                nc.scalar.activation(out=est[md][:, s, :], in_=acc, func=Act.Exp)
        # denom (column) + reciprocal
        rd = pA.tile([P, NS, NCH], f32, tag="rdch", name="rdch")
        for s in range(NS):
            for mc in range(NCH):
                acc = ps.tile([P, 1], f32, tag="bank", name="bank")
                for kd in range(NCH):
                    mm(acc, lhsT=est[kd][:, s, mc*P:(mc+1)*P], rhs=ones_col_bf,
                       start=(kd == 0), stop=(kd == NCH-1))
                nc.vector.reciprocal(out=rd[:, s, mc:mc+1], in_=acc)
        # vals = E^T v, normalized by 1/denom per row
        vals = []
        for mc in range(NCH):
            vt = pA.tile([P, NS, L], bf16, tag=f"vals{mc}", name=f"vals{mc}")
            for s in range(NS):
                acc = ps.tile([P, L], f32, tag="bank", name="bank")
                for kd in range(NCH):
                    mm(acc, lhsT=est[kd][:, s, mc*P:(mc+1)*P], rhs=vch[kd][:, s, :],
                       start=(kd == 0), stop=(kd == NCH-1))
                nc.vector.tensor_scalar(out=vt[:, s, :], in0=acc,
                                        scalar1=rd[:, s, mc:mc+1],
                                        scalar2=None, op0=Alu.mult)
            vals.append(vt)
        # x_down2 = x_down + outch @ vals + b ; GN2 partials
        xd2 = [ppL.tile([P, NS, L], f32, tag=f"xd2_{mo}", name=f"xd2_{mo}") for mo in range(NCH)]
        p2 = [ppL.tile([P, 4], f32, tag=f"p2_{mo}", name=f"p2_{mo}") for mo in range(NCH)]
        for mo in range(NCH):
            for s in range(NS):
                acc = ps.tile([P, L], f32, tag="bank", name="bank")
                for cc in range(NCH):
                    mm(acc, lhsT=wsb['outch'][cc][:, mo*P:(mo+1)*P],
                       rhs=vals[cc][:, s, :], start=(cc == 0), stop=(cc == NCH-1))
                sq = sq2.tile([P, L], f32, tag="sqscratch", name="sqscratch")
                nc.vector.scalar_tensor_tensor(out=xd2[mo][:, s, :], in0=acc,
                                               scalar=wsb['outchb'][mo][:, 0:1],
                                               in1=xd[mo][:, s, :], op0=Alu.add, op1=Alu.add,
                                               accum_out=p2[mo][:, 2*s:2*s+1])
                nc.scalar.activation(out=sq, in_=xd2[mo][:, s, :], func=Act.Square,
                                     accum_out=p2[mo][:, 2*s+1:2*s+2])

    if stage == 'ch':
        for cc in range(NCH):
            for s in range(NS):
                nc.sync.dma_start(out=io['out'][s, cc*P:(cc+1)*P], in_=xd2[cc][:, s, :])
        wABstack.close()
        return

    # ============================================================ phase B2: spatial attention
    # up-conv diagonals for cc0/cc1 are built here, while Pool is idle, so
    # phase C's first matmul isn't gated on an affine_select
    sCstack = ExitStack()
    sC = sCstack.enter_context(tc.tile_pool(name="sC", bufs=2))
    updiags = {}

    def mk_updiag(cc):
        # 65 columns: cols 0..63 are the conv taps, col 64 is the identity
        # (residual) tap
        if v2:
            updiags[cc] = wsb['updiagh'][cc]
            return
        ud = sC.tile([P, 65, P], bf16, tag="diag", name=f"updiag{cc}")
        nc.gpsimd.affine_select(
            out=ud, in_=wsb['upw'][cc].unsqueeze(2).broadcast_to([P, 65, P]),
            compare_op=Alu.is_equal, fill=0.0, base=0,
            pattern=[[0, 65], [1, P]], channel_multiplier=-1)
        updiags[cc] = ud

    mk_updiag(0)
    mk_updiag(1)

    with ExitStack() as ctxB:
        pB = ctxB.enter_context(tc.tile_pool(name="pB", bufs=1))
        sB = ctxB.enter_context(tc.tile_pool(name="sB", bufs=3))


        mean2, r2 = gn_stats(pB, p2, "gn2")
        xn2 = []
        for cc in range(NCH):
            xn = pB.tile([P, NS, L], bf16, tag=f"xn2_{cc}", name=f"xn2_{cc}")
            for s in range(NS):
                nc.vector.tensor_scalar(out=xn[:, s, :], in0=xd2[cc][:, s, :],
                                        scalar1=mean2[:, s:s+1], scalar2=r2[:, s:s+1],
                                        op0=Alu.subtract, op1=Alu.mult)
            xn2.append(xn)

        # q,k channel-major [2C, NS*L], bf16 (halves SBUF; logits accumulate f32)
        qksp = []
        for mq in range(2*NCH):
            qt = pB.tile([P, NS, L], bf16, tag=f"qksp{mq}", name=f"qksp{mq}")
            for s in range(NS):
                acc = ps.tile([P, L], f32, tag="bank", name="bank")
                for cc in range(NCH):
                    mm(acc, lhsT=wsb['wqksp'][cc][:, mq*P:(mq+1)*P],
                       rhs=xn2[cc][:, s, :], start=(cc == 0), stop=(cc == NCH-1))
                nc.vector.tensor_scalar(out=qt[:, s, :], in0=acc,
                                        scalar1=wsb['wbqksp'][mq][:, 0:1], scalar2=None, op0=Alu.add)
            qksp.append(qt)
        # v^T pix-major [L, C] per sample, bf16
        vsp = [pB.tile([P, NS, C], bf16, tag=f"vsp{ml}", name=f"vsp{ml}") for ml in range(2)]
        for s in range(NS):
            for ml in range(2):
                acc = ps.tile([P, C], f32, tag="bank", name="bank")
                for cc in range(NCH):
                    mm(acc, lhsT=xn2[cc][:, s, ml*P:(ml+1)*P], rhs=wsb['wvsp'][cc],
                       start=(cc == 0), stop=False)
                mmr(acc, lhsT=ones_row, rhs=wsb['wbvsp_row'], start=False, stop=True)
                nc.scalar.copy(out=vsp[ml][:, s, :], in_=acc)
        # softmax+vals pipelined at 2-head granularity: spps/dvdb alternate the
        # two PSUM slots, so the next pair's logit matmuls overlap this pair's
        # exp/normalize
        valssp = pB.tile([32, HEADS, NS, L], bf16, tag="valssp", name="valssp")
        # x3 output tiles + per-sample projection: emitting the projection right
        # after each sample's softmax keeps PE fed during the other sample's
        # exp/normalize bubbles
        x3p = [ppL.tile([P, NS, 18, 18], bf16, tag=f"x3p{mo}", name=f"x3p{mo}")
               for mo in range(NCH)]
        for mo in range(NCH):
            nc.vector.memset(x3p[mo], 0.0)
        ospw = wsb['outsp'].rearrange("p (h c) -> p h c", h=HEADS)

        def proj_x3(s):
            for mo in range(NCH):
                acc = ps.tile([P, L], f32, tag="bank", name="bank")
                for h in range(HEADS):
                    mm(acc, lhsT=ospw[:, h, mo*P:(mo+1)*P],
                       rhs=valssp[:, h, s, :],
                       start=(h == 0), stop=(h == HEADS-1))
                nc.vector.scalar_tensor_tensor(
                    out=x3p[mo][:, s, 1:17, 1:17],
                    in0=acc.rearrange("p (a b) -> p a b", a=16),
                    scalar=wsb['outspb'][mo][:, 0:1],
                    in1=xd2[mo][:, s, :].rearrange("p (a b) -> p a b", a=16),
                    op0=Alu.add, op1=Alu.add)

        def b2_logits(s, hb, half):
            spps = ps.tile([P, 4, L], f32, tag="bank", name="spps")
            for mk in range(2):
                for hx in range(2):
                    hh = half*2 + hx
                    h = hb*4 + hh
                    po = hh*32
                    mm(spps[:, hx*2+mk, :],
                       lhsT=qksp[NCH + h//4][po:po+32, s, mk*P:(mk+1)*P],
                       rhs=qksp[h//4][po:po+32, s, :],
                       start=True, stop=True, tile_position=(po, 0))
            esp = sB.tile([P, 4, L], bf16, tag="esp", name="esp")
            nc.scalar.activation(out=esp, in_=spps, func=Act.Exp)
            return esp

        def b2_vals(s, hb, half, esp):
            # complete each plane's accumulation group before the next --
            # interleaved groups sharing a PSUM bank lose the first write
            dvdb = ps.tile([32, 4, L], f32, tag="bank", name="dvdb")
            for hx in range(2):
                h = hb*4 + half*2 + hx
                for mk in range(2):
                    mm(dvdb[:, hx, :], lhsT=vsp[mk][:, s, h*32:(h+1)*32],
                       rhs=esp[:, hx*2+mk, :], start=(mk == 0), stop=(mk == 1))
            for hx in range(2):
                for mk in range(2):
                    mm(dvdb[:, 2+hx, :], lhsT=ones32, rhs=esp[:, hx*2+mk, :],
                       start=(mk == 0), stop=(mk == 1))
            rec = sB.tile([32, 2, L], f32, tag="recsp", name="recsp")
            nc.vector.reciprocal(out=rec, in_=dvdb[:, 2:4, :])
            for hx in range(2):
                nc.vector.tensor_tensor(
                    out=valssp[:, hb*4+half*2+hx, s, :],
                    in0=dvdb[:, hx, :], in1=rec[:, hx, :], op=Alu.mult)

        if v4:
            # samples interleaved + lookahead-2: PE always has the next two
            # iterations' logit matmuls queued ahead of the dvdb group that
            # waits on this iteration's exp
            seq = [(s, hb, half) for hb in range(NCH) for half in range(2)
                   for s in range(NS)]
            D = 2
            esp_q = {}
            for i in range(D):
                esp_q[i] = b2_logits(*seq[i])
            for i, it in enumerate(seq):
                if i + D < len(seq):
                    esp_q[i + D] = b2_logits(*seq[i + D])
                b2_vals(*it, esp_q.pop(i))
            proj_x3(0)
            proj_x3(1)
        else:
            for s in range(NS):
                for hb in range(NCH):
                    for half in range(2):
                        b2_vals(s, hb, half, b2_logits(s, hb, half))
                proj_x3(s)

    if stage == 'valssp':
        for h in range(HEADS):
            for s in range(NS):
                nc.sync.dma_start(out=io['out'][s, h*32:(h+1)*32, :],
                                  in_=valssp[:, h, s, :])
        return

    if stage == 'sp':
        for cc in range(NCH):
            for s in range(NS):
                cv = ppL.tile([P, 16, 16], f32, tag="spcv", name="spcv")
                nc.scalar.copy(out=cv, in_=x3p[cc][:, s, 1:17, 1:17])
                nc.sync.dma_start(
                    out=io['out'][s, cc*P:(cc+1)*P].rearrange("p (a b) -> p a b", a=16),
                    in_=cv)
        sCstack.close()
        wABstack.close()
        return

    # ============================================================ phase C: up conv + residual
    # residual added via the identity tap (updiag col 64) against the strided
    # bf16 x view — PE handles strided rhs reads fine, so no ofull prefill.
    # Evictions alternate DVE / Act so the strided writes split across engines.
    with ExitStack() as ctxC:
        sCo = ctxC.enter_context(tc.tile_pool(name="sCo", bufs=3))
        mk_updiag(2)
        for cc in range(NCH):
            updiag = updiags[cc]
            xbf4 = xbf[cc][:, :, Z:Z+4096].rearrange("p s (h w) -> p s h w", h=64)
            ofull = [sCo.tile([P, 4096], f32, tag="ofull", name=f"ofull{cc}_{s}")
                     for s in range(NS)]
            of4 = [o.rearrange("p (a b) -> p a b", a=64) for o in ofull]
            for r1 in range(4):
                for r2 in range(4):
                    ph = r1*4 + r2
                    acc = ps.tile([P, NS, 16, 16], f32, tag="bank", name="upps")
                    for t in range(4):
                        j1, j2 = upoff[ph][t]
                        mm(acc, lhsT=updiag[:, ph*4+t, :],
                           rhs=x3p[cc][:, :, 1+j1:17+j1, 1+j2:17+j2],
                           start=(t == 0), stop=False)
                    mm(acc, lhsT=updiag[:, 64, :],
                       rhs=xbf4[:, :, r1::4, r2::4], start=False, stop=True)
                    for s in range(NS):
                        ov = of4[s][:, r1::4, r2::4]
                        if (ph + s) % 2 == 0:
                            nc.vector.tensor_scalar(
                                out=ov, in0=acc[:, s],
                                scalar1=wsb['upb'][cc][:, 0:1], scalar2=None,
                                op0=Alu.add)
                        else:
                            nc.scalar.add(out=ov, in_=acc[:, s],
                                          add=wsb['upb'][cc][:, 0:1])
            for s in range(NS):
                (nc.scalar.dma_start if v3 else nc.sync.dma_start)(
                    out=io['out'][s, cc*P:(cc+1)*P].rearrange("c h w -> c (h w)"),
                    in_=ofull[s][:, :])
    sCstack.close()
    wABstack.close()


STAGE_OUT_SHAPE = {
    'dw':  [NS, C, L], 'gn1': [NS, C, L], 'ch': [NS, C, L], 'sp': [NS, C, L],
    'valssp': [NS, C, L], 'full': [NS, C, 64, 64],
    'selonly': [NS, C, 64, 64], 'conly': [NS, C, 64, 64], 'noop': [NS, C, 64, 64], 'noop2': [NS, C, 64, 64], 'mmbig': [NS, C, 64, 64], 'mmsmall': [NS, C, 64, 64],
    'fullv2': [NS, C, 64, 64], 'chv2': [NS, C, L], 'spv2': [NS, C, L],
    'fullv3': [NS, C, 64, 64], 'fullv4': [NS, C, 64, 64],
}


def build(d, stage='full', reps=1):
    import ml_dtypes
    nc = bacc.Bacc("TRN2", target_bir_lowering=False, debug=False, num_devices=NCORES)
    io = {}
    io['x'] = nc.declare_dram_parameter("x", [NS, C, 64, 64], f32, isOutput=False)[:]
    for k in DEV_KEYS:
        dt = bf16 if d[k].dtype == ml_dtypes.bfloat16 else f32
        io[k] = nc.declare_dram_parameter(k, list(d[k].shape), dt, isOutput=False)[:]
    io['out'] = nc.declare_dram_parameter("out", STAGE_OUT_SHAPE[stage], f32, isOutput=True)[:]
    with ExitStack() as ctx:
        tc = ctx.enter_context(tile.TileContext(nc))
        for _ in range(reps):
            with ExitStack() as rctx:
                emit(nc, tc, rctx, io, d['upoff'], stage)
    nc.finalize()
    return nc


# ----------------------------------------------------------------------------- runner
class Runner:
    """Builds once; keeps a persistent jitted executable for repeat (timed) runs."""

    def __init__(self, d, stage='full', reps=1):
        self.d = d
        self.stage = stage
        self.nc = build(d, stage, reps)
        self._fn = None

    def _make_fn(self):
        import jax
        from jax.sharding import Mesh, PartitionSpec
        from jax.experimental.shard_map import shard_map
        from concourse.bass2jax import _bass_exec_p, install_neuronx_cc_hook, partition_id_tensor
        install_neuronx_cc_hook()
        nc = self.nc
        part_name = nc.partition_id_tensor.name if nc.partition_id_tensor else None
        in_names, out_names, out_avals, zero_outs = [], [], [], []
        for alloc in nc.m.functions[0].allocations:
            if not isinstance(alloc, mybir.MemoryLocationSet):
                continue
            name = alloc.memorylocations[0].name
            if alloc.kind == "ExternalInput":
                if name != part_name:
                    in_names.append(name)
            elif alloc.kind == "ExternalOutput":
                shape = tuple(alloc.tensor_shape)
                dtype = mybir.dt.np(alloc.dtype)
                out_names.append(name)
                out_avals.append(jax.core.ShapedArray(shape, dtype))
                zero_outs.append(np.zeros(shape, dtype))
        self.in_names = in_names
        self.out_names = out_names
        self.zero_outs = zero_outs
        n_params = len(in_names)
        all_names = in_names + out_names
        if part_name is not None:
            all_names = all_names + [part_name]

        def _body(*args):
            operands = list(args)
            if part_name is not None:
                operands.append(partition_id_tensor())
            outs = _bass_exec_p.bind(
                *operands, out_avals=tuple(out_avals), in_names=tuple(all_names),
                out_names=tuple(out_names), lowering_input_output_aliases=(),
                sim_require_finite=False, sim_require_nnan=False, nc=nc)
            return tuple(outs)

        devices = jax.devices()[:NCORES]
        mesh = Mesh(np.asarray(devices), ("core",))
        self._mesh = mesh
        donate = tuple(range(n_params, n_params + len(out_names)))
        self._fn = jax.jit(
            shard_map(_body, mesh=mesh,
                      in_specs=(PartitionSpec("core"),) * (n_params + len(out_names)),
                      out_specs=(PartitionSpec("core"),) * len(out_names)),
            donate_argnums=donate, keep_unused=True)
        self._out_shapes = [tuple(a.shape) for a in out_avals]

    def run(self, in_maps, timeit=False, reps=5):
        import jax, time
        from jax.sharding import NamedSharding, PartitionSpec
        if self._fn is None:
            self._make_fn()
        concat_in = [np.concatenate([np.asarray(m[name]) for m in in_maps], axis=0)
                     for name in self.in_names]
        concat_zero = [np.zeros((NCORES * z.shape[0], *z.shape[1:]), z.dtype)
                       for z in self.zero_outs]
        out = self._fn(*concat_in, *concat_zero)
        jax.block_until_ready(out)
        tmin = None
        if timeit:
            sh = NamedSharding(self._mesh, PartitionSpec("core"))
            dev_in = [jax.device_put(a, sh) for a in concat_in]
            jax.block_until_ready(dev_in)
            times = []
            for _ in range(reps):
                cz = [jax.device_put(np.zeros_like(z), sh) for z in concat_zero]
                jax.block_until_ready(cz)
                t0 = time.perf_counter()
                o2 = self._fn(*dev_in, *cz)
                jax.block_until_ready(o2)
                times.append(time.perf_counter() - t0)
            tmin = min(times)
            import os
            if os.environ.get('PRINT_TIMES'):
                print('times_ms:', [round(t*1e3,2) for t in times])
        outs = [np.asarray(o).reshape(NCORES, *self._out_shapes[i])
                for i, o in enumerate(out)]
        per_core = [{name: outs[i][c] for i, name in enumerate(self.out_names)}
                    for c in range(NCORES)]
        return per_core, tmin


_RUNNER_CACHE = {}


def _get_runner(d, stage='full'):
    if stage not in _RUNNER_CACHE:
        _RUNNER_CACHE[stage] = Runner(d, stage)
    return _RUNNER_CACHE[stage]


def make_in_maps(d, x):
    """x: [16, C, 64, 64] -> per-core input maps."""
    maps = []
    for c in range(NCORES):
        m = {k: d[k] for k in DEV_KEYS}
        m['x'] = np.ascontiguousarray(x[c*NS:(c+1)*NS])
        maps.append(m)
    return maps


def kernel(**inputs):
    inputs = {k: np.asarray(v) for k, v in inputs.items()}
    d = host_precompute(inputs)
    r = _get_runner(d, 'fullv4')
    res, _ = r.run(make_in_maps(d, inputs['x']))
    return np.concatenate([res[c]['out'] for c in range(NCORES)], axis=0)

